# revision 24
# baseline (speedup 1.0000x reference)
"""Trainium2 Bass kernel for masked attention-weight computation.

Reference computation:
    proj    = einsum('lbh,gh->lbg', enc, W) + bias          # Linear
    energies= einsum('lbg,bg->bl', proj, hidden[0])         # [B,L]
    energies= where(l < lengths[b], energies, -1e20)
    out     = renormalized masked softmax(energies)         # [B,1,L]

Algebraic restructure: energies[b,l] = enc[l,b,:] . u[b,:] + c[b] with
u = hidden @ W and c[b] = hidden[b] . bias.  c[b] is constant per row so it
cancels in softmax; the invalid positions are -1e20 either way.  So the
device kernel only needs u (tiny matmul) plus a batched dot product over
enc (memory bound), then a masked softmax.

Sharding: data-parallel over batch. 8 cores x 4 batches each. Each core
streams its 32MB enc shard once; W is replicated.
"""

import contextlib
import ctypes
import os
import sys
import types

import numpy as np

import concourse.bacc as bacc
from concourse import mybir
from concourse.tile import TileContext
from concourse.bass_utils import run_bass_kernel_spmd

L, B, H = 2048, 32, 1024
NCORES = 8
BS = B // NCORES          # batches per core
NT = L // 128             # l-tiles of 128
GC = H // 128             # g-chunks for the u matmul
FLOAT_MIN = -1e20
F32 = mybir.dt.float32

_NC_CACHE = {}

# filled by the most recent kernel() call when BASS_TRACE=1
last_exec_time_ns = None
last_trace_path = None


def _install_ntff_hook():
    """Provide antenv.axon_hooks (missing in this image) so that
    run_bass_kernel_spmd(trace=True) can capture NTFF profiles via the
    axon PJRT side channel.  No-op if already present."""
    if "antenv.axon_hooks" in sys.modules:
        return
    so_path = "/opt/axon/libaxon_pjrt.so"
    hook = None
    if os.path.exists(so_path):
        try:
            lib = ctypes.CDLL(so_path)
            if hasattr(lib, "axon_start_nrt_profile"):
                lib.axon_start_nrt_profile.argtypes = [
                    ctypes.POINTER(ctypes.c_int64), ctypes.c_size_t]
                lib.axon_start_nrt_profile.restype = ctypes.c_int64
                lib.axon_stop_nrt_profile.argtypes = [ctypes.c_char_p]
                lib.axon_stop_nrt_profile.restype = ctypes.c_int64

                @contextlib.contextmanager
                def _hookfn(output_dir, device_ids):
                    import jax
                    jax.devices()
                    if device_ids:
                        ids = (ctypes.c_int64 * len(device_ids))(*device_ids)
                        rc = lib.axon_start_nrt_profile(ids, len(device_ids))
                    else:
                        rc = lib.axon_start_nrt_profile(None, 0)
                    if rc != 0:
                        raise RuntimeError(f"axon_start_nrt_profile rc={rc}")
                    try:
                        yield
                    finally:
                        lib.axon_stop_nrt_profile(str(output_dir).encode())

                hook = _hookfn
        except OSError:
            hook = None
    mod = types.ModuleType("antenv.axon_hooks")
    mod.get_axon_ntff_profile_hook = lambda: hook
    mod.set_axon_ntff_profile_hook = lambda h: None
    sys.modules["antenv.axon_hooks"] = mod
    try:
        import antenv
        antenv.axon_hooks = mod
    except ImportError:
        pass


_install_ntff_hook()


def _build():
    nc = bacc.Bacc()
    BF16 = mybir.dt.bfloat16
    enc = nc.declare_dram_parameter("enc", [L, BS, H], F32, isOutput=False)
    # hidden^T and W in bf16 hi/lo pairs: x = hi + lo to ~2^-17 relative, so
    # u = hT_hi@W_hi + hT_hi@W_lo + hT_lo@W_hi runs at bf16 matmul speed with
    # near-fp32 accuracy (fp32 PSUM accumulation).
    hT = nc.declare_dram_parameter("hT", [128, 2 * GC * BS], BF16, isOutput=False)
    Wp = nc.declare_dram_parameter("W", [2, H, H], BF16, isOutput=False)
    pen = nc.declare_dram_parameter("pen", [128, BS * NT], F32, isOutput=False)
    ident = nc.declare_dram_parameter("ident", [128, 128], F32, isOutput=False)
    # sel[b] is a [BS, 128] matrix whose row b is all-ones: lhsT for the
    # TensorE partition-broadcast of u (exact in fp32).
    sel = nc.declare_dram_parameter("sel", [BS, BS, 128], F32, isOutput=False)
    outp = nc.declare_dram_parameter("out", [BS, L], F32, isOutput=True)

    with TileContext(nc) as tc:
        with (
            tc.tile_pool(name="const", bufs=1) as constp,
            tc.tile_pool(name="wpool", bufs=8) as wpool,
            tc.tile_pool(name="encp", bufs=24) as encp,
            tc.tile_pool(name="scrp", bufs=2) as scrp,
            tc.tile_pool(name="small", bufs=1) as small,
            tc.tile_pool(name="psum", bufs=1, space="PSUM") as psum,
        ):
            ident_sb = constp.tile([128, 128], F32, tag="ident")
            nc.sync.dma_start(out=ident_sb, in_=ident[:, :])
            pen_sb = constp.tile([128, BS * NT], F32, tag="pen")
            nc.sync.dma_start(out=pen_sb, in_=pen[:, :])
            # hT_sb[p, pass, gc, b]: pass 0 = hi, pass 1 = lo
            hT_sb = constp.tile([128, 2, GC, BS], BF16, tag="hT")
            nc.sync.dma_start(out=hT_sb, in_=hT[:, :])

            # ---- u = hidden @ W  -> [BS, H]  (3-pass bf16 hi/lo) ----
            u_ps0 = psum.tile([BS, 512], F32, tag="ups0")
            u_ps1 = psum.tile([BS, 512], F32, tag="ups1")
            n_mm = 3 * GC
            i_mm = 0
            for gc in range(GC):
                wt_hi = wpool.tile([128, H], BF16, tag="wt_hi", name="wt_hi")
                nc.sync.dma_start(out=wt_hi, in_=Wp[0, gc * 128:(gc + 1) * 128, :])
                wt_lo = wpool.tile([128, H], BF16, tag="wt_lo", name="wt_lo")
                nc.sync.dma_start(out=wt_lo, in_=Wp[1, gc * 128:(gc + 1) * 128, :])
                for lhs_pass, wt in ((0, wt_hi), (0, wt_lo), (1, wt_hi)):
                    lhsT = hT_sb[:, lhs_pass, gc, :]
                    nc.tensor.matmul(
                        u_ps0[:, :], lhsT=lhsT, rhs=wt[:, 0:512],
                        start=(i_mm == 0), stop=(i_mm == n_mm - 1))
                    nc.tensor.matmul(
                        u_ps1[:, :], lhsT=lhsT, rhs=wt[:, 512:1024],
                        start=(i_mm == 0), stop=(i_mm == n_mm - 1))
                    i_mm += 1

            u_sb = small.tile([BS, H], F32, tag="u_sb")
            nc.scalar.copy(u_sb[:, 0:512], u_ps0[:, :])
            nc.scalar.copy(u_sb[:, 512:1024], u_ps1[:, :])

            # broadcast u across partitions on TensorE:
            #   u_bc[p, h] = sum_b' sel[b][b', p] * u_sb[b', h]   (= u[b, h])
            sel_sb = constp.tile([BS, BS, 128], F32, tag="sel")
            nc.sync.dma_start(out=sel_sb, in_=sel[:, :, :])
            u_bc = constp.tile([128, BS, H], F32, tag="u_bc")
            for b in range(BS):
                bc_ps = psum.tile([128, H], F32, tag="bc_ps", name="bc_ps",
                                  bufs=2)
                for ns in range(2):
                    nc.tensor.matmul(
                        bc_ps[:, ns * 512:(ns + 1) * 512],
                        lhsT=sel_sb[:, b, :],
                        rhs=u_sb[:, ns * 512:(ns + 1) * 512],
                        start=True, stop=True)
                nc.scalar.copy(u_bc[:, b, :], bc_ps[:, :])

            # ---- energies, in two half-tiles so the reorientation of the
            # first half overlaps the second half of the main loop ----
            # Eh[p, b*HT + t] = enc[(h*HT+t)*128+p, b, :] . u[b, :]
            HT = NT // 2
            E_half = [small.tile([128, BS * HT], F32, tag=f"E{h}", name=f"E{h}")
                      for h in range(2)]
            e4 = small.tile([BS, L], F32, tag="e4")
            m1 = [small.tile([128, BS], F32, tag=f"m1_{h}", name=f"m1_{h}") for h in range(2)]
            et_sb = [small.tile([BS * HT, 128], F32, tag=f"et_sb{h}", name=f"et_sb{h}")
                     for h in range(2)]
            for half in range(2):
                E_sb = E_half[half]
                for th in range(HT):
                    t = half * HT + th
                    for b in range(BS):
                        c = b * HT + th
                        piece = encp.tile([128, H], F32, tag="enc")
                        nc.sync.dma_start(out=piece,
                                          in_=enc[t * 128:(t + 1) * 128, b, :])
                        scr = scrp.tile([128, H], F32, tag="scr")
                        nc.vector.scalar_tensor_tensor(
                            out=scr,
                            in0=piece[:, :],
                            scalar=1.0,
                            in1=u_bc[:, b, :],
                            op0=mybir.AluOpType.mult,
                            op1=mybir.AluOpType.mult,
                            accum_out=E_sb[:, c:c + 1],
                        )
                # mask this half: pen_sb viewed as [128, (b, half, th)]
                pen_v = pen_sb[:, :].rearrange("p (b g t) -> p b g t", b=BS, g=2)
                nc.vector.tensor_tensor(
                    out=E_sb[:, :].rearrange("p (b t) -> p b t", b=BS),
                    in0=E_sb[:, :].rearrange("p (b t) -> p b t", b=BS),
                    in1=pen_v[:, :, half, :], op=mybir.AluOpType.add)
                # per-(l%128) max over this half's t, per batch
                nc.vector.reduce_max(
                    out=m1[half][:, :],
                    in_=E_sb[:, :].rearrange("p (b t) -> p b t", b=BS),
                    axis=mybir.AxisListType.X)
                # reorient this half: [128, BS*HT] -> [BS*HT, 128] -> [BS, L/2]
                et_ps = psum.tile([BS * HT, 128], F32, tag="et")
                nc.tensor.transpose(et_ps[:, :], E_sb[:, :], ident_sb[:, :])
                nc.scalar.copy(et_sb[half][:, :], et_ps[:, :])
                nc.scalar.dma_start(
                    out=e4[:, :].rearrange("b (g t p) -> b g t p", g=2, t=HT)
                    [:, half, :, :],
                    in_=et_sb[half][:, :])

            # ---- masked softmax over free dim ----
            # global max: combine the two half-maxes, transpose [128,BS] ->
            # [BS,128], reduce (negated) -> [BS,1]
            m1c = small.tile([128, BS], F32, tag="m1c")
            nc.vector.tensor_tensor(out=m1c[:, :], in0=m1[0][:, :],
                                    in1=m1[1][:, :], op=mybir.AluOpType.max)
            mt_ps = psum.tile([BS, 128], F32, tag="mt")
            nc.tensor.transpose(mt_ps[:, :], m1c[:, :], ident_sb[:, :])
            negmx = small.tile([BS, 1], F32, tag="negmx")
            nc.vector.reduce_max(out=negmx, in_=mt_ps[:, :],
                                 axis=mybir.AxisListType.X, negate=True)
            p4 = small.tile([BS, L], F32, tag="p4")
            s4 = small.tile([BS, 1], F32, tag="s4")
            nc.scalar.activation(
                out=p4, in_=e4[:, :],
                func=mybir.ActivationFunctionType.Exp,
                bias=negmx[:, 0:1], scale=1.0, accum_out=s4)
            r4 = small.tile([BS, 1], F32, tag="r4")
            nc.vector.reciprocal(r4, s4)
            nc.vector.tensor_scalar(p4[:, :], p4[:, :], r4[:, 0:1], None,
                                    mybir.AluOpType.mult)
            nc.scalar.dma_start(out=outp[:, :], in_=p4[:, :])

    nc.compile()
    return nc


def get_nc():
    if "nc" not in _NC_CACHE:
        _NC_CACHE["nc"] = _build()
    return _NC_CACHE["nc"]


def _to_bf16_pair(x):
    """fp32 array -> stacked [2, ...] uint16-viewed bf16 hi/lo parts."""
    import jax.numpy as jnp
    x = np.asarray(x, np.float32)
    hi = np.asarray(jnp.asarray(x).astype(jnp.bfloat16))
    lo = np.asarray((jnp.asarray(x) - jnp.asarray(hi).astype(jnp.float32)
                     ).astype(jnp.bfloat16))
    return hi, lo


def _host_prep(hidden, encoder_outputs, W, lengths):
    """Build the per-core input maps (pure layout work, no math)."""
    hidden = np.asarray(hidden, dtype=np.float32)
    encoder_outputs = np.asarray(encoder_outputs, dtype=np.float32)
    lengths = np.asarray(lengths)
    ident = np.eye(128, dtype=np.float32)
    # sel[b', b, p] = 1 iff b' == b  (TensorE partition-broadcast selector)
    sel = np.ascontiguousarray(
        np.broadcast_to(np.eye(BS, dtype=np.float32)[:, :, None],
                        (BS, BS, 128)))
    h_hi, h_lo = _to_bf16_pair(hidden[0])                         # [B, H] bf16
    W_hi, W_lo = _to_bf16_pair(W)
    W_pair = np.ascontiguousarray(np.stack([W_hi, W_lo], axis=0))  # [2, H, H]
    in_maps = []
    for c in range(NCORES):
        sl = slice(c * BS, (c + 1) * BS)
        # hT[p, pass, gc, b] = hidden_pass[b, gc*128 + p]
        hT = np.stack([
            np.asarray(part[sl]).reshape(BS, GC, 128).transpose(2, 1, 0)
            for part in (h_hi, h_lo)], axis=1)                    # [128,2,GC,BS]
        hT = np.ascontiguousarray(hT.reshape(128, 2 * GC * BS))
        enc_sh = np.ascontiguousarray(encoder_outputs[:, sl, :])  # [L, BS, H]
        len_sh = lengths[sl].astype(np.int64)
        # pen[p, b*NT + t] = 0 if t*128+p < len[b] else FLOAT_MIN
        pos = np.arange(L).reshape(NT, 128)                       # [t, p]
        valid = pos[None, :, :] < len_sh[:, None, None]           # [b, t, p]
        pen = np.where(valid, 0.0, FLOAT_MIN).astype(np.float32)  # [b, t, p]
        pen = np.ascontiguousarray(pen.transpose(2, 0, 1).reshape(128, BS * NT))
        in_maps.append({"enc": enc_sh, "hT": hT, "pen": pen, "ident": ident,
                        "sel": sel, "W": W_pair})
    return in_maps


def kernel(hidden, encoder_outputs, W, b, lengths):
    global last_exec_time_ns, last_trace_path
    nc = get_nc()
    in_maps = _host_prep(hidden, encoder_outputs, W, lengths)
    trace = bool(os.environ.get("BASS_TRACE"))
    res = run_bass_kernel_spmd(nc, in_maps, core_ids=list(range(NCORES)),
                               trace=trace)
    last_exec_time_ns = res.exec_time_ns
    it = res.instructions_and_trace
    last_trace_path = it[1] if it else None
    out = np.concatenate([res.results[c]["out"] for c in range(NCORES)], axis=0)
    return out.reshape(B, 1, L).astype(np.float32)


# revision 26
# speedup vs baseline: 1.1826x; 1.1826x over previous
"""Trainium2 Bass kernel for masked attention-weight computation.

Reference computation:
    proj    = einsum('lbh,gh->lbg', enc, W) + bias          # Linear
    energies= einsum('lbg,bg->bl', proj, hidden[0])         # [B,L]
    energies= where(l < lengths[b], energies, -1e20)
    out     = renormalized masked softmax(energies)         # [B,1,L]

Algebraic restructure: energies[b,l] = enc[l,b,:] . u[b,:] + c[b] with
u = hidden @ W and c[b] = hidden[b] . bias.  c[b] is constant per row so it
cancels in softmax; the invalid positions are -1e20 either way.  So the
device kernel only needs u (tiny matmul) plus a batched dot product over
enc (memory bound), then a masked softmax.

Sharding: data-parallel over batch. 8 cores x 4 batches each. Each core
streams its 32MB enc shard once; W is replicated.
"""

import contextlib
import ctypes
import os
import sys
import types

import numpy as np

import concourse.bacc as bacc
from concourse import mybir
from concourse.tile import TileContext
from concourse.bass_utils import run_bass_kernel_spmd

L, B, H = 2048, 32, 1024
NCORES = 8
BS = B // NCORES          # batches per core
NT = L // 128             # l-tiles of 128
GC = H // 128             # g-chunks for the u matmul
FLOAT_MIN = -1e20
F32 = mybir.dt.float32

_NC_CACHE = {}

# filled by the most recent kernel() call when BASS_TRACE=1
last_exec_time_ns = None
last_trace_path = None


def _install_ntff_hook():
    """Provide antenv.axon_hooks (missing in this image) so that
    run_bass_kernel_spmd(trace=True) can capture NTFF profiles via the
    axon PJRT side channel.  No-op if already present."""
    if "antenv.axon_hooks" in sys.modules:
        return
    so_path = "/opt/axon/libaxon_pjrt.so"
    hook = None
    if os.path.exists(so_path):
        try:
            lib = ctypes.CDLL(so_path)
            if hasattr(lib, "axon_start_nrt_profile"):
                lib.axon_start_nrt_profile.argtypes = [
                    ctypes.POINTER(ctypes.c_int64), ctypes.c_size_t]
                lib.axon_start_nrt_profile.restype = ctypes.c_int64
                lib.axon_stop_nrt_profile.argtypes = [ctypes.c_char_p]
                lib.axon_stop_nrt_profile.restype = ctypes.c_int64

                @contextlib.contextmanager
                def _hookfn(output_dir, device_ids):
                    import jax
                    jax.devices()
                    if device_ids:
                        ids = (ctypes.c_int64 * len(device_ids))(*device_ids)
                        rc = lib.axon_start_nrt_profile(ids, len(device_ids))
                    else:
                        rc = lib.axon_start_nrt_profile(None, 0)
                    if rc != 0:
                        raise RuntimeError(f"axon_start_nrt_profile rc={rc}")
                    try:
                        yield
                    finally:
                        lib.axon_stop_nrt_profile(str(output_dir).encode())

                hook = _hookfn
        except OSError:
            hook = None
    mod = types.ModuleType("antenv.axon_hooks")
    mod.get_axon_ntff_profile_hook = lambda: hook
    mod.set_axon_ntff_profile_hook = lambda h: None
    sys.modules["antenv.axon_hooks"] = mod
    try:
        import antenv
        antenv.axon_hooks = mod
    except ImportError:
        pass


_install_ntff_hook()


def _build():
    nc = bacc.Bacc()
    BF16 = mybir.dt.bfloat16
    enc = nc.declare_dram_parameter("enc", [L, BS, H], F32, isOutput=False)
    # hidden^T and W in bf16 hi/lo pairs: x = hi + lo to ~2^-17 relative, so
    # u = hT_hi@W_hi + hT_hi@W_lo + hT_lo@W_hi runs at bf16 matmul speed with
    # near-fp32 accuracy (fp32 PSUM accumulation).
    hT = nc.declare_dram_parameter("hT", [128, 2 * GC * BS], BF16, isOutput=False)
    Wp = nc.declare_dram_parameter("W", [2, H, H], BF16, isOutput=False)
    pen = nc.declare_dram_parameter("pen", [128, BS * NT], F32, isOutput=False)
    ident = nc.declare_dram_parameter("ident", [128, 128], F32, isOutput=False)
    # sel[b] is a [BS, 128] matrix whose row b is all-ones: lhsT for the
    # TensorE partition-broadcast of u (exact in fp32).
    sel = nc.declare_dram_parameter("sel", [BS, BS, 128], F32, isOutput=False)
    outp = nc.declare_dram_parameter("out", [BS, L], F32, isOutput=True)

    with TileContext(nc) as tc:
        with (
            tc.tile_pool(name="const", bufs=1) as constp,
            tc.tile_pool(name="wpool", bufs=8) as wpool,
            tc.tile_pool(name="encp", bufs=12) as encp,
            tc.tile_pool(name="scrp", bufs=2) as scrp,
            tc.tile_pool(name="small", bufs=1) as small,
            tc.tile_pool(name="psum", bufs=1, space="PSUM") as psum,
        ):
            ident_sb = constp.tile([128, 128], F32, tag="ident")
            nc.sync.dma_start(out=ident_sb, in_=ident[:, :])
            pen_sb = constp.tile([128, BS * NT], F32, tag="pen")
            nc.sync.dma_start(out=pen_sb, in_=pen[:, :])
            # hT_sb[p, pass, gc, b]: pass 0 = hi, pass 1 = lo
            hT_sb = constp.tile([128, 2, GC, BS], BF16, tag="hT")
            nc.sync.dma_start(out=hT_sb, in_=hT[:, :])

            # ---- u = hidden @ W  -> [BS, H]  (3-pass bf16 hi/lo) ----
            u_ps0 = psum.tile([BS, 512], F32, tag="ups0")
            u_ps1 = psum.tile([BS, 512], F32, tag="ups1")
            n_mm = 3 * GC
            i_mm = 0
            for gc in range(GC):
                wt_hi = wpool.tile([128, H], BF16, tag="wt_hi", name="wt_hi")
                nc.sync.dma_start(out=wt_hi, in_=Wp[0, gc * 128:(gc + 1) * 128, :])
                wt_lo = wpool.tile([128, H], BF16, tag="wt_lo", name="wt_lo")
                nc.sync.dma_start(out=wt_lo, in_=Wp[1, gc * 128:(gc + 1) * 128, :])
                for lhs_pass, wt in ((0, wt_hi), (0, wt_lo), (1, wt_hi)):
                    lhsT = hT_sb[:, lhs_pass, gc, :]
                    nc.tensor.matmul(
                        u_ps0[:, :], lhsT=lhsT, rhs=wt[:, 0:512],
                        start=(i_mm == 0), stop=(i_mm == n_mm - 1))
                    nc.tensor.matmul(
                        u_ps1[:, :], lhsT=lhsT, rhs=wt[:, 512:1024],
                        start=(i_mm == 0), stop=(i_mm == n_mm - 1))
                    i_mm += 1

            u_sb = small.tile([BS, H], F32, tag="u_sb")
            nc.scalar.copy(u_sb[:, 0:512], u_ps0[:, :])
            nc.scalar.copy(u_sb[:, 512:1024], u_ps1[:, :])

            # broadcast u across partitions on TensorE:
            #   u_bc[p, h] = sum_b' sel[b][b', p] * u_sb[b', h]   (= u[b, h])
            sel_sb = constp.tile([BS, BS, 128], F32, tag="sel")
            nc.sync.dma_start(out=sel_sb, in_=sel[:, :, :])
            u_bc = constp.tile([128, BS, H], F32, tag="u_bc")
            for b in range(BS):
                bc_ps = psum.tile([128, H], F32, tag="bc_ps", name="bc_ps",
                                  bufs=2)
                for ns in range(2):
                    nc.tensor.matmul(
                        bc_ps[:, ns * 512:(ns + 1) * 512],
                        lhsT=sel_sb[:, b, :],
                        rhs=u_sb[:, ns * 512:(ns + 1) * 512],
                        start=True, stop=True)
                nc.scalar.copy(u_bc[:, b, :], bc_ps[:, :])

            # ---- energies, in two half-tiles so the reorientation of the
            # first half overlaps the second half of the main loop ----
            # Eh[p, b*HT + t] = enc[(h*HT+t)*128+p, b, :] . u[b, :]
            HT = NT // 2
            E_half = [small.tile([128, BS * HT], F32, tag=f"E{h}", name=f"E{h}")
                      for h in range(2)]
            e4 = small.tile([BS, L], F32, tag="e4")
            m1 = [small.tile([128, BS], F32, tag=f"m1_{h}", name=f"m1_{h}") for h in range(2)]
            et_sb = [small.tile([BS * HT, 128], F32, tag=f"et_sb{h}", name=f"et_sb{h}")
                     for h in range(2)]
            for half in range(2):
                E_sb = E_half[half]
                for th in range(HT):
                    t = half * HT + th
                    for bp in range(BS // 2):
                        piece = encp.tile([128, 2, H], F32, tag="enc")
                        nc.sync.dma_start(
                            out=piece,
                            in_=enc[t * 128:(t + 1) * 128, 2 * bp:2 * bp + 2, :])
                        for j in range(2):
                            b = 2 * bp + j
                            c = b * HT + th
                            scr = scrp.tile([128, H], F32, tag="scr")
                            nc.vector.scalar_tensor_tensor(
                                out=scr,
                                in0=piece[:, j, :],
                                scalar=1.0,
                                in1=u_bc[:, b, :],
                                op0=mybir.AluOpType.mult,
                                op1=mybir.AluOpType.mult,
                                accum_out=E_sb[:, c:c + 1],
                            )
                # mask this half: pen_sb viewed as [128, (b, half, th)]
                pen_v = pen_sb[:, :].rearrange("p (b g t) -> p b g t", b=BS, g=2)
                nc.vector.tensor_tensor(
                    out=E_sb[:, :].rearrange("p (b t) -> p b t", b=BS),
                    in0=E_sb[:, :].rearrange("p (b t) -> p b t", b=BS),
                    in1=pen_v[:, :, half, :], op=mybir.AluOpType.add)
                # per-(l%128) max over this half's t, per batch
                nc.vector.reduce_max(
                    out=m1[half][:, :],
                    in_=E_sb[:, :].rearrange("p (b t) -> p b t", b=BS),
                    axis=mybir.AxisListType.X)
                # reorient this half: [128, BS*HT] -> [BS*HT, 128] -> [BS, L/2]
                et_ps = psum.tile([BS * HT, 128], F32, tag="et")
                nc.tensor.transpose(et_ps[:, :], E_sb[:, :], ident_sb[:, :])
                nc.scalar.copy(et_sb[half][:, :], et_ps[:, :])
                nc.scalar.dma_start(
                    out=e4[:, :].rearrange("b (g t p) -> b g t p", g=2, t=HT)
                    [:, half, :, :],
                    in_=et_sb[half][:, :])

            # ---- masked softmax over free dim ----
            # global max: combine the two half-maxes, transpose [128,BS] ->
            # [BS,128], reduce (negated) -> [BS,1]
            m1c = small.tile([128, BS], F32, tag="m1c")
            nc.vector.tensor_tensor(out=m1c[:, :], in0=m1[0][:, :],
                                    in1=m1[1][:, :], op=mybir.AluOpType.max)
            mt_ps = psum.tile([BS, 128], F32, tag="mt")
            nc.tensor.transpose(mt_ps[:, :], m1c[:, :], ident_sb[:, :])
            negmx = small.tile([BS, 1], F32, tag="negmx")
            nc.vector.reduce_max(out=negmx, in_=mt_ps[:, :],
                                 axis=mybir.AxisListType.X, negate=True)
            p4 = small.tile([BS, L], F32, tag="p4")
            s4 = small.tile([BS, 1], F32, tag="s4")
            nc.scalar.activation(
                out=p4, in_=e4[:, :],
                func=mybir.ActivationFunctionType.Exp,
                bias=negmx[:, 0:1], scale=1.0, accum_out=s4)
            r4 = small.tile([BS, 1], F32, tag="r4")
            nc.vector.reciprocal(r4, s4)
            nc.vector.tensor_scalar(p4[:, :], p4[:, :], r4[:, 0:1], None,
                                    mybir.AluOpType.mult)
            nc.scalar.dma_start(out=outp[:, :], in_=p4[:, :])

    nc.compile()
    return nc


def get_nc():
    if "nc" not in _NC_CACHE:
        _NC_CACHE["nc"] = _build()
    return _NC_CACHE["nc"]


def _to_bf16_pair(x):
    """fp32 array -> stacked [2, ...] uint16-viewed bf16 hi/lo parts."""
    import jax.numpy as jnp
    x = np.asarray(x, np.float32)
    hi = np.asarray(jnp.asarray(x).astype(jnp.bfloat16))
    lo = np.asarray((jnp.asarray(x) - jnp.asarray(hi).astype(jnp.float32)
                     ).astype(jnp.bfloat16))
    return hi, lo


def _host_prep(hidden, encoder_outputs, W, lengths):
    """Build the per-core input maps (pure layout work, no math)."""
    hidden = np.asarray(hidden, dtype=np.float32)
    encoder_outputs = np.asarray(encoder_outputs, dtype=np.float32)
    lengths = np.asarray(lengths)
    ident = np.eye(128, dtype=np.float32)
    # sel[b', b, p] = 1 iff b' == b  (TensorE partition-broadcast selector)
    sel = np.ascontiguousarray(
        np.broadcast_to(np.eye(BS, dtype=np.float32)[:, :, None],
                        (BS, BS, 128)))
    h_hi, h_lo = _to_bf16_pair(hidden[0])                         # [B, H] bf16
    W_hi, W_lo = _to_bf16_pair(W)
    W_pair = np.ascontiguousarray(np.stack([W_hi, W_lo], axis=0))  # [2, H, H]
    in_maps = []
    for c in range(NCORES):
        sl = slice(c * BS, (c + 1) * BS)
        # hT[p, pass, gc, b] = hidden_pass[b, gc*128 + p]
        hT = np.stack([
            np.asarray(part[sl]).reshape(BS, GC, 128).transpose(2, 1, 0)
            for part in (h_hi, h_lo)], axis=1)                    # [128,2,GC,BS]
        hT = np.ascontiguousarray(hT.reshape(128, 2 * GC * BS))
        enc_sh = np.ascontiguousarray(encoder_outputs[:, sl, :])  # [L, BS, H]
        len_sh = lengths[sl].astype(np.int64)
        # pen[p, b*NT + t] = 0 if t*128+p < len[b] else FLOAT_MIN
        pos = np.arange(L).reshape(NT, 128)                       # [t, p]
        valid = pos[None, :, :] < len_sh[:, None, None]           # [b, t, p]
        pen = np.where(valid, 0.0, FLOAT_MIN).astype(np.float32)  # [b, t, p]
        pen = np.ascontiguousarray(pen.transpose(2, 0, 1).reshape(128, BS * NT))
        in_maps.append({"enc": enc_sh, "hT": hT, "pen": pen, "ident": ident,
                        "sel": sel, "W": W_pair})
    return in_maps


def kernel(hidden, encoder_outputs, W, b, lengths):
    global last_exec_time_ns, last_trace_path
    nc = get_nc()
    in_maps = _host_prep(hidden, encoder_outputs, W, lengths)
    trace = bool(os.environ.get("BASS_TRACE"))
    res = run_bass_kernel_spmd(nc, in_maps, core_ids=list(range(NCORES)),
                               trace=trace)
    last_exec_time_ns = res.exec_time_ns
    it = res.instructions_and_trace
    last_trace_path = it[1] if it else None
    out = np.concatenate([res.results[c]["out"] for c in range(NCORES)], axis=0)
    return out.reshape(B, 1, L).astype(np.float32)


# revision 29
# speedup vs baseline: 1.1991x; 1.0140x over previous
"""Trainium2 Bass kernel for masked attention-weight computation.

Reference computation:
    proj    = einsum('lbh,gh->lbg', enc, W) + bias          # Linear
    energies= einsum('lbg,bg->bl', proj, hidden[0])         # [B,L]
    energies= where(l < lengths[b], energies, -1e20)
    out     = renormalized masked softmax(energies)         # [B,1,L]

Algebraic restructure: energies[b,l] = enc[l,b,:] . u[b,:] + c[b] with
u = hidden @ W and c[b] = hidden[b] . bias.  c[b] is constant per row so it
cancels in softmax; the invalid positions are -1e20 either way.  So the
device kernel only needs u (tiny matmul) plus a batched dot product over
enc (memory bound), then a masked softmax.

Sharding: data-parallel over batch. 8 cores x 4 batches each. Each core
streams its 32MB enc shard once; W is replicated.
"""

import contextlib
import ctypes
import os
import sys
import types

import numpy as np

import concourse.bacc as bacc
from concourse import mybir
from concourse.tile import TileContext
from concourse.bass_utils import run_bass_kernel_spmd

L, B, H = 2048, 32, 1024
NCORES = 8
BS = B // NCORES          # batches per core
NT = L // 128             # l-tiles of 128
GC = H // 128             # g-chunks for the u matmul
FLOAT_MIN = -1e20
F32 = mybir.dt.float32

_NC_CACHE = {}

# filled by the most recent kernel() call when BASS_TRACE=1
last_exec_time_ns = None
last_trace_path = None


def _install_ntff_hook():
    """Provide antenv.axon_hooks (missing in this image) so that
    run_bass_kernel_spmd(trace=True) can capture NTFF profiles via the
    axon PJRT side channel.  No-op if already present."""
    if "antenv.axon_hooks" in sys.modules:
        return
    so_path = "/opt/axon/libaxon_pjrt.so"
    hook = None
    if os.path.exists(so_path):
        try:
            lib = ctypes.CDLL(so_path)
            if hasattr(lib, "axon_start_nrt_profile"):
                lib.axon_start_nrt_profile.argtypes = [
                    ctypes.POINTER(ctypes.c_int64), ctypes.c_size_t]
                lib.axon_start_nrt_profile.restype = ctypes.c_int64
                lib.axon_stop_nrt_profile.argtypes = [ctypes.c_char_p]
                lib.axon_stop_nrt_profile.restype = ctypes.c_int64

                @contextlib.contextmanager
                def _hookfn(output_dir, device_ids):
                    import jax
                    jax.devices()
                    if device_ids:
                        ids = (ctypes.c_int64 * len(device_ids))(*device_ids)
                        rc = lib.axon_start_nrt_profile(ids, len(device_ids))
                    else:
                        rc = lib.axon_start_nrt_profile(None, 0)
                    if rc != 0:
                        raise RuntimeError(f"axon_start_nrt_profile rc={rc}")
                    try:
                        yield
                    finally:
                        lib.axon_stop_nrt_profile(str(output_dir).encode())

                hook = _hookfn
        except OSError:
            hook = None
    mod = types.ModuleType("antenv.axon_hooks")
    mod.get_axon_ntff_profile_hook = lambda: hook
    mod.set_axon_ntff_profile_hook = lambda h: None
    sys.modules["antenv.axon_hooks"] = mod
    try:
        import antenv
        antenv.axon_hooks = mod
    except ImportError:
        pass


_install_ntff_hook()


def _build():
    nc = bacc.Bacc()
    BF16 = mybir.dt.bfloat16
    enc = nc.declare_dram_parameter("enc", [L, BS, H], F32, isOutput=False)
    # hidden^T and W in bf16 hi/lo pairs: x = hi + lo to ~2^-17 relative, so
    # u = hT_hi@W_hi + hT_hi@W_lo + hT_lo@W_hi runs at bf16 matmul speed with
    # near-fp32 accuracy (fp32 PSUM accumulation).
    hT = nc.declare_dram_parameter("hT", [128, 2 * GC * BS], BF16, isOutput=False)
    Wp = nc.declare_dram_parameter("W", [2, H, H], BF16, isOutput=False)
    pen = nc.declare_dram_parameter("pen", [128, BS * NT], F32, isOutput=False)
    ident = nc.declare_dram_parameter("ident", [128, 128], F32, isOutput=False)
    # sel[b] is a [BS, 128] matrix whose row b is all-ones: lhsT for the
    # TensorE partition-broadcast of u (exact in fp32).
    sel = nc.declare_dram_parameter("sel", [BS, BS, 128], F32, isOutput=False)
    sel32 = nc.declare_dram_parameter("sel32", [BS, BS * (NT // 2)], F32,
                                      isOutput=False)
    sel32t = nc.declare_dram_parameter("sel32t", [BS * (NT // 2), BS], F32,
                                       isOutput=False)
    outp = nc.declare_dram_parameter("out", [BS, L], F32, isOutput=True)

    with TileContext(nc) as tc:
        with (
            tc.tile_pool(name="const", bufs=1) as constp,
            tc.tile_pool(name="wpool", bufs=8) as wpool,
            tc.tile_pool(name="encp", bufs=12) as encp,
            tc.tile_pool(name="scrp", bufs=2) as scrp,
            tc.tile_pool(name="small", bufs=1) as small,
            tc.tile_pool(name="psum", bufs=1, space="PSUM") as psum,
        ):
            ident_sb = constp.tile([128, 128], F32, tag="ident")
            nc.sync.dma_start(out=ident_sb, in_=ident[:, :])
            pen_sb = constp.tile([128, BS * NT], F32, tag="pen")
            nc.sync.dma_start(out=pen_sb, in_=pen[:, :])
            # hT_sb[p, pass, gc, b]: pass 0 = hi, pass 1 = lo
            hT_sb = constp.tile([128, 2, GC, BS], BF16, tag="hT")
            nc.sync.dma_start(out=hT_sb, in_=hT[:, :])

            # ---- u = hidden @ W  -> [BS, H]  (3-pass bf16 hi/lo) ----
            u_ps0 = psum.tile([BS, 512], F32, tag="ps_small", bufs=4)
            u_ps1 = psum.tile([BS, 512], F32, tag="ps_small", bufs=4)
            n_mm = 3 * GC
            i_mm = 0
            for gc in range(GC):
                wt_hi = wpool.tile([128, H], BF16, tag="wt_hi", name="wt_hi")
                nc.sync.dma_start(out=wt_hi, in_=Wp[0, gc * 128:(gc + 1) * 128, :])
                wt_lo = wpool.tile([128, H], BF16, tag="wt_lo", name="wt_lo")
                nc.sync.dma_start(out=wt_lo, in_=Wp[1, gc * 128:(gc + 1) * 128, :])
                for lhs_pass, wt in ((0, wt_hi), (0, wt_lo), (1, wt_hi)):
                    lhsT = hT_sb[:, lhs_pass, gc, :]
                    nc.tensor.matmul(
                        u_ps0[:, :], lhsT=lhsT, rhs=wt[:, 0:512],
                        start=(i_mm == 0), stop=(i_mm == n_mm - 1))
                    nc.tensor.matmul(
                        u_ps1[:, :], lhsT=lhsT, rhs=wt[:, 512:1024],
                        start=(i_mm == 0), stop=(i_mm == n_mm - 1))
                    i_mm += 1

            u_sb = small.tile([BS, H], F32, tag="u_sb")
            nc.scalar.copy(u_sb[:, 0:512], u_ps0[:, :])
            nc.scalar.copy(u_sb[:, 512:1024], u_ps1[:, :])

            # broadcast u across partitions on TensorE:
            #   u_bc[p, h] = sum_b' sel[b][b', p] * u_sb[b', h]   (= u[b, h])
            sel_sb = constp.tile([BS, BS, 128], F32, tag="sel")
            nc.sync.dma_start(out=sel_sb, in_=sel[:, :, :])
            u_bc = constp.tile([128, BS, H], F32, tag="u_bc")
            for b in range(BS):
                bc_ps = psum.tile([128, H], F32, tag="ps_big", name="bc_ps",
                                  bufs=2)
                for ns in range(2):
                    nc.tensor.matmul(
                        bc_ps[:, ns * 512:(ns + 1) * 512],
                        lhsT=sel_sb[:, b, :],
                        rhs=u_sb[:, ns * 512:(ns + 1) * 512],
                        start=True, stop=True)
                nc.scalar.copy(u_bc[:, b, :], bc_ps[:, :])

            # ---- energies, in two half-tiles so the reorientation of the
            # first half overlaps the second half of the main loop ----
            # Eh[p, b*HT + t] = enc[(h*HT+t)*128+p, b, :] . u[b, :]
            HT = NT // 2
            E_half = [small.tile([128, BS * HT], F32, tag=f"E{h}", name=f"E{h}")
                      for h in range(2)]
            m1 = [small.tile([128, BS], F32, tag=f"m1_{h}", name=f"m1_{h}") for h in range(2)]
            et_sb = [small.tile([BS * HT, 128], F32, tag=f"et_sb{h}", name=f"et_sb{h}")
                     for h in range(2)]
            for half in range(2):
                E_sb = E_half[half]
                for th in range(HT):
                    t = half * HT + th
                    for bp in range(BS // 2):
                        piece = encp.tile([128, 2, H], F32, tag="enc")
                        nc.sync.dma_start(
                            out=piece,
                            in_=enc[t * 128:(t + 1) * 128, 2 * bp:2 * bp + 2, :])
                        for j in range(2):
                            b = 2 * bp + j
                            c = b * HT + th
                            scr = scrp.tile([128, H], F32, tag="scr")
                            nc.vector.scalar_tensor_tensor(
                                out=scr,
                                in0=piece[:, j, :],
                                scalar=1.0,
                                in1=u_bc[:, b, :],
                                op0=mybir.AluOpType.mult,
                                op1=mybir.AluOpType.mult,
                                accum_out=E_sb[:, c:c + 1],
                            )
                # mask this half: pen_sb viewed as [128, (b, half, th)]
                pen_v = pen_sb[:, :].rearrange("p (b g t) -> p b g t", b=BS, g=2)
                nc.vector.tensor_tensor(
                    out=E_sb[:, :].rearrange("p (b t) -> p b t", b=BS),
                    in0=E_sb[:, :].rearrange("p (b t) -> p b t", b=BS),
                    in1=pen_v[:, :, half, :], op=mybir.AluOpType.add)
                # per-(l%128) max over this half's t, per batch
                nc.vector.reduce_max(
                    out=m1[half][:, :],
                    in_=E_sb[:, :].rearrange("p (b t) -> p b t", b=BS),
                    axis=mybir.AxisListType.X)
                # reorient this half: [128, BS*HT] -> [BS*HT, 128] -> [BS, L/2]
                et_ps = psum.tile([BS * HT, 128], F32, tag="ps_big", name="et_ps", bufs=2)
                nc.tensor.transpose(et_ps[:, :], E_sb[:, :], ident_sb[:, :])
                nc.scalar.copy(et_sb[half][:, :], et_ps[:, :])

            # ---- masked softmax, entirely in the [BS*HT, 128] layout ----
            # global max: combine the two half-maxes, transpose [128,BS] ->
            # [BS,128], reduce (negated) -> [BS,1]
            m1c = small.tile([128, BS], F32, tag="m1c")
            nc.vector.tensor_tensor(out=m1c[:, :], in0=m1[0][:, :],
                                    in1=m1[1][:, :], op=mybir.AluOpType.max)
            mt_ps = psum.tile([BS, 128], F32, tag="ps_small", bufs=4)
            nc.tensor.transpose(mt_ps[:, :], m1c[:, :], ident_sb[:, :])
            negmx = small.tile([BS, 1], F32, tag="negmx")
            nc.vector.reduce_max(out=negmx, in_=mt_ps[:, :],
                                 axis=mybir.AxisListType.X, negate=True)
            # bias32[q] = negmx[q // HT] via selector matmul (exact)
            sel32_sb = constp.tile([BS, BS * HT], F32, tag="sel32")
            nc.sync.dma_start(out=sel32_sb, in_=sel32[:, :])
            sel32t_sb = constp.tile([BS * HT, BS], F32, tag="sel32t")
            nc.sync.dma_start(out=sel32t_sb, in_=sel32t[:, :])
            b32_ps = psum.tile([BS * HT, 1], F32, tag="ps_small", bufs=4)
            nc.tensor.matmul(b32_ps[:, :], lhsT=sel32_sb[:, :],
                             rhs=negmx[:, 0:1], start=True, stop=True)
            bias32 = small.tile([BS * HT, 1], F32, tag="bias32")
            nc.scalar.copy(bias32[:, :], b32_ps[:, :])
            # exp + per-partition sums, each half
            s32 = small.tile([BS * HT, 2], F32, tag="s32")
            for half in range(2):
                nc.scalar.activation(
                    out=et_sb[half][:, :], in_=et_sb[half][:, :],
                    func=mybir.ActivationFunctionType.Exp,
                    bias=bias32[:, 0:1], scale=1.0,
                    accum_out=s32[:, half:half + 1])
            s32c = small.tile([BS * HT, 1], F32, tag="s32c")
            nc.vector.tensor_tensor(out=s32c[:, :], in0=s32[:, 0:1],
                                    in1=s32[:, 1:2], op=mybir.AluOpType.add)
            # S[b] = sum over the b's 16 partitions, then 1/S, re-expanded
            S_ps = psum.tile([BS, 1], F32, tag="ps_small", bufs=4)
            nc.tensor.matmul(S_ps[:, :], lhsT=sel32t_sb[:, :], rhs=s32c[:, :],
                             start=True, stop=True)
            r4 = small.tile([BS, 1], F32, tag="r4")
            nc.vector.reciprocal(r4, S_ps[:, :])
            r32_ps = psum.tile([BS * HT, 1], F32, tag="ps_small", bufs=4)
            nc.tensor.matmul(r32_ps[:, :], lhsT=sel32_sb[:, :], rhs=r4[:, :],
                             start=True, stop=True)
            r32_sb = small.tile([BS * HT, 1], F32, tag="r32sb")
            nc.scalar.copy(r32_sb[:, :], r32_ps[:, :])
            # scale and write out; the DRAM write does the [b*HT+t, p] ->
            # [b, t*128+p] reorientation
            outv = outp[:, :].rearrange("b (g t p) -> b g t p", g=2, t=HT)
            for half in range(2):
                nc.vector.tensor_scalar(et_sb[half][:, :], et_sb[half][:, :],
                                        r32_sb[:, 0:1], None,
                                        mybir.AluOpType.mult)
                nc.scalar.dma_start(out=outv[:, half, :, :],
                                    in_=et_sb[half][:, :])

    nc.compile()
    return nc


def get_nc():
    if "nc" not in _NC_CACHE:
        _NC_CACHE["nc"] = _build()
    return _NC_CACHE["nc"]


def _to_bf16_pair(x):
    """fp32 array -> stacked [2, ...] uint16-viewed bf16 hi/lo parts."""
    import jax.numpy as jnp
    x = np.asarray(x, np.float32)
    hi = np.asarray(jnp.asarray(x).astype(jnp.bfloat16))
    lo = np.asarray((jnp.asarray(x) - jnp.asarray(hi).astype(jnp.float32)
                     ).astype(jnp.bfloat16))
    return hi, lo


def _host_prep(hidden, encoder_outputs, W, lengths):
    """Build the per-core input maps (pure layout work, no math)."""
    hidden = np.asarray(hidden, dtype=np.float32)
    encoder_outputs = np.asarray(encoder_outputs, dtype=np.float32)
    lengths = np.asarray(lengths)
    ident = np.eye(128, dtype=np.float32)
    # sel[b', b, p] = 1 iff b' == b  (TensorE partition-broadcast selector)
    sel = np.ascontiguousarray(
        np.broadcast_to(np.eye(BS, dtype=np.float32)[:, :, None],
                        (BS, BS, 128)))
    HT = NT // 2
    sel32t = np.ascontiguousarray(
        np.repeat(np.eye(BS, dtype=np.float32), HT, axis=0))   # [BS*HT, BS]
    sel32 = np.ascontiguousarray(sel32t.T)                     # [BS, BS*HT]
    h_hi, h_lo = _to_bf16_pair(hidden[0])                         # [B, H] bf16
    W_hi, W_lo = _to_bf16_pair(W)
    W_pair = np.ascontiguousarray(np.stack([W_hi, W_lo], axis=0))  # [2, H, H]
    in_maps = []
    for c in range(NCORES):
        sl = slice(c * BS, (c + 1) * BS)
        # hT[p, pass, gc, b] = hidden_pass[b, gc*128 + p]
        hT = np.stack([
            np.asarray(part[sl]).reshape(BS, GC, 128).transpose(2, 1, 0)
            for part in (h_hi, h_lo)], axis=1)                    # [128,2,GC,BS]
        hT = np.ascontiguousarray(hT.reshape(128, 2 * GC * BS))
        enc_sh = np.ascontiguousarray(encoder_outputs[:, sl, :])  # [L, BS, H]
        len_sh = lengths[sl].astype(np.int64)
        # pen[p, b*NT + t] = 0 if t*128+p < len[b] else FLOAT_MIN
        pos = np.arange(L).reshape(NT, 128)                       # [t, p]
        valid = pos[None, :, :] < len_sh[:, None, None]           # [b, t, p]
        pen = np.where(valid, 0.0, FLOAT_MIN).astype(np.float32)  # [b, t, p]
        pen = np.ascontiguousarray(pen.transpose(2, 0, 1).reshape(128, BS * NT))
        in_maps.append({"enc": enc_sh, "hT": hT, "pen": pen, "ident": ident,
                        "sel": sel, "sel32": sel32, "sel32t": sel32t,
                        "W": W_pair})
    return in_maps


def kernel(hidden, encoder_outputs, W, b, lengths):
    global last_exec_time_ns, last_trace_path
    nc = get_nc()
    in_maps = _host_prep(hidden, encoder_outputs, W, lengths)
    trace = bool(os.environ.get("BASS_TRACE"))
    res = run_bass_kernel_spmd(nc, in_maps, core_ids=list(range(NCORES)),
                               trace=trace)
    last_exec_time_ns = res.exec_time_ns
    it = res.instructions_and_trace
    last_trace_path = it[1] if it else None
    out = np.concatenate([res.results[c]["out"] for c in range(NCORES)], axis=0)
    return out.reshape(B, 1, L).astype(np.float32)
